# revision 7
# baseline (speedup 1.0000x reference)
"""HGAT message-passing kernel for Trainium2 (8 NeuronCores, SPMD).

Reference computation (B=4, N=4096, C_IN=128, C_OUT=64):
    h   = node_rep @ proj_W.T + proj_b                    # [B,N,64]
    f1  = rowsum(h * k_W[node_type]) + k_b[node_type]     # [B,N]
    f2  = rowsum(h * v_W[node_type]) + v_b[node_type]     # [B,N]
    L   = adj[i,j] * (f1[i] + f2[j])
    u   = sigmoid(L) - 0.5
    P   = softmax(u, axis=i)      # normalized over rows i, per column j
    out = P @ h                   # contract over j

Key algebra used on device:
  * softmax-over-i / contract-over-j means out = E @ (h / colsum) with
    E[i,j] = exp(sigmoid(L)) and colsum[j] = sum_i E[i,j]; the -0.5 and the
    softmax max-subtraction cancel in the ratio.
  * exp(sigmoid(x)) is itself sigmoid-shaped (range (1, e)); the fit
    E ~= D + A*sigmoid(B*x + C) has max rel err 4.1e-4 over all x, so ONE
    ACT pass (Sigmoid, accum_out -> colsum) replaces the tanh+exp pair.
    The B scale folds into the host-prescaled k/v params, C is the ACT
    bias, A and the rank-1 D-term (D * sum_j g[j,:]) fold out on the host
    (which already sums the per-half partial outputs).
  * final matmul is transposed (out_T[o,i] = g.T @ s'), so the PE streams
    512-column chunks with a 64-column weight load instead of the reverse.
  * adjacency travels as bf16 (tolerance 2e-2; quantization adds ~1e-4).

Sharding: core c handles batch b=c//2 and j-half h=c%2 (rows of adj.T).
Host pre-transposes/casts adj, gathers+prescales k_W/v_W rows by node_type
(pure data movement), and combines A*(M0+M1).T + D*(Sg0+Sg1) at the end.
"""

import os
import sys

import numpy as np

sys.path.insert(0, "/opt/trn_rl_repo")

import ml_dtypes  # noqa: E402

import concourse.tile as tile  # noqa: E402
from concourse import bacc  # noqa: E402
from concourse import mybir  # noqa: E402
from concourse.bass_utils import run_bass_kernel_spmd  # noqa: E402

B = 4
N = 4096
CIN = 128
COUT = 64
P = 128                      # SBUF partitions
NJ = N // 2                  # j rows per core (adjacency half)
NJT = NJ // P                # 16 j-tiles per core
NIC = N // 512               # 8 i-chunks of 512

F32 = mybir.dt.float32
F32R = mybir.dt.float32r
BF16 = mybir.dt.bfloat16
AF = mybir.ActivationFunctionType
ALU = mybir.AluOpType

# exp(sigmoid(x)) ~= FIT_D + FIT_A * sigmoid(FIT_B * x + FIT_C)
FIT_A = 1.71677394
FIT_B = 1.01816816
FIT_C = -0.49959447
FIT_D = 1.00040553

LAST_EXEC_NS = None
LAST_RESULTS = None


def build_nc():
    """Single-core SPMD Bass program (same program on all cores)."""
    nc = bacc.Bacc()
    adjt_d = nc.dram_tensor("adjt", [NJ, N], BF16, kind="ExternalInput")
    xt_d = nc.dram_tensor("xt", [CIN, N], BF16, kind="ExternalInput")
    xth_d = nc.dram_tensor("xth", [CIN, NJ], BF16, kind="ExternalInput")
    wpt_d = nc.dram_tensor("wpt", [CIN, COUT], BF16, kind="ExternalInput")
    bpcol_d = nc.dram_tensor("bpcol", [COUT, 1], F32, kind="ExternalInput")
    bpb_d = nc.dram_tensor("bpb", [P, COUT], F32, kind="ExternalInput")
    kwt_d = nc.dram_tensor("kwt", [COUT, N], BF16, kind="ExternalInput")
    kbrow_d = nc.dram_tensor("kbrow", [1, N], BF16, kind="ExternalInput")
    vwn_d = nc.dram_tensor("vwn", [P, NJT * COUT], BF16, kind="ExternalInput")
    vbcol_d = nc.dram_tensor("vbcol", [P, NJT], F32, kind="ExternalInput")
    ones65_d = nc.dram_tensor("ones65", [COUT + 1, P], BF16, kind="ExternalInput")
    outp_d = nc.dram_tensor("outp", [COUT, N], F32, kind="ExternalOutput")
    gout_d = nc.dram_tensor("gout", [P, NJT * COUT], F32R, kind="ExternalOutput")

    with tile.TileContext(nc) as tc:
        with (
            tc.tile_pool(name="singles", bufs=1) as singles,
            tc.tile_pool(name="stream", bufs=3) as stream,
            tc.tile_pool(name="adjp", bufs=3) as adjp,
            tc.tile_pool(name="lp", bufs=2) as lp,
            tc.tile_pool(name="etp", bufs=2) as etp,
            tc.tile_pool(name="smalls", bufs=3) as smalls,
        ):
            # ---------------- small parameter loads ----------------
            wpt_s = singles.tile([CIN, COUT], BF16)
            nc.sync.dma_start(wpt_s, wpt_d[:, :])
            bpcol_s = singles.tile([COUT, 1], F32)
            nc.sync.dma_start(bpcol_s, bpcol_d[:, :])
            bpb_s = singles.tile([P, COUT], F32)
            nc.sync.dma_start(bpb_s, bpb_d[:, :])
            vbcol_s = singles.tile([P, NJT], F32)
            nc.sync.dma_start(vbcol_s, vbcol_d[:, :])
            ones65 = singles.tile([COUT + 1, P], BF16)
            nc.sync.dma_start(ones65, ones65_d[:, :])
            cbias = singles.tile([P, 1], F32)
            nc.vector.memset(cbias, FIT_C)

            f1b = singles.tile([P, N], BF16)
            hn = singles.tile([P, NJT * COUT], F32)
            f2c = singles.tile([P, NJT], F32)
            f2cb = singles.tile([P, NJT], BF16)
            g_all = singles.tile([P, NJT * COUT], F32R)

            # ---------------- pre-phase (PSUM pools scoped) ----------------
            with (
                tc.tile_pool(name="psPreA", bufs=2, space="PSUM") as psA,
                tc.tile_pool(name="psPreB", bufs=2, space="PSUM") as psB,
            ):
                # f1 row (prescaled by FIT_B via kwt/kbrow), streamed in
                # 512-col chunks; broadcast across partitions via K=1 matmul.
                # Per chunk: hT, then one K=65 matmul against all-ones [65,P]
                # does reduce-over-o + kb add (row 64) + broadcast to all
                # partitions in a single instruction.
                for ic in range(NIC):
                    sl = slice(ic * 512, (ic + 1) * 512)
                    xtc = stream.tile([CIN, 512], BF16, tag="xtc")
                    nc.sync.dma_start(xtc, xt_d[:, sl])
                    psh = psA.tile([COUT, 512], F32, tag="psh")
                    nc.tensor.matmul(psh, lhsT=wpt_s, rhs=xtc, start=True, stop=True)
                    kwc = stream.tile([COUT, 512], BF16, tag="kwc")
                    nc.sync.dma_start(kwc, kwt_d[:, sl])
                    prod = stream.tile([COUT + 1, 512], BF16, tag="prod")
                    nc.sync.dma_start(prod[COUT:COUT + 1, :], kbrow_d[:, sl])
                    nc.vector.scalar_tensor_tensor(
                        prod[0:COUT, :], psh, bpcol_s, kwc, op0=ALU.add, op1=ALU.mult
                    )
                    psb = psA.tile([P, 512], F32, tag="psb")
                    nc.tensor.matmul(psb, lhsT=ones65, rhs=prod, start=True, stop=True)
                    nc.scalar.copy(f1b[:, sl], psb)

                # h natural (j-half nodes) + f2, per 128-node tile
                for t in range(NJT):
                    osl = slice(t * COUT, (t + 1) * COUT)
                    xthc = stream.tile([CIN, P], BF16, tag="xthc")
                    nc.sync.dma_start(xthc, xth_d[:, t * P:(t + 1) * P])
                    psn = psB.tile([P, COUT], F32, tag="psn")
                    nc.tensor.matmul(psn, lhsT=xthc, rhs=wpt_s, start=True, stop=True)
                    nc.vector.tensor_add(hn[:, osl], psn, bpb_s)
                    vwc = stream.tile([P, COUT], BF16, tag="vwc")
                    nc.sync.dma_start(vwc, vwn_d[:, osl])
                    pvc = stream.tile([P, COUT], F32, tag="pvc")
                    nc.vector.tensor_mul(pvc, hn[:, osl], vwc)
                    nc.vector.tensor_reduce(
                        f2c[:, t:t + 1], pvc, axis=mybir.AxisListType.X, op=ALU.add
                    )
                nc.vector.tensor_add(f2cb, f2c, vbcol_s)

            # ---------------- main loop over j-tiles ----------------
            with tc.tile_pool(name="psMain", bufs=1, space="PSUM") as psM:
                ps_out = psM.tile([COUT, N], F32)

                def post_sigma(jt, cs, sp):
                    # colsum = D*N + A*acc ; g = h / colsum ; out_T += g.T @ s'
                    t1 = smalls.tile([P, 1], F32, tag="t1")
                    nc.vector.tensor_scalar(
                        t1, cs, FIT_A, float(FIT_D * N), op0=ALU.mult, op1=ALU.add
                    )
                    rc = smalls.tile([P, 1], F32, tag="rc")
                    nc.vector.reciprocal(rc, t1)
                    gsl = slice(jt * COUT, (jt + 1) * COUT)
                    nc.vector.tensor_scalar_mul(g_all[:, gsl], hn[:, gsl], rc)
                    for c in range(NIC):
                        csl = slice(c * 512, (c + 1) * 512)
                        nc.tensor.matmul(
                            ps_out[:, csl],
                            lhsT=g_all[:, gsl],
                            rhs=sp[:, csl],
                            start=(jt == 0),
                            stop=(jt == NJT - 1),
                        )

                # Software pipeline: tile jt's post-sigma DVE work is emitted
                # AFTER tile jt+1's stt, so the DVE never stalls waiting on
                # the ACT sigmoid of the current tile.
                pend = None
                for jt in range(NJT):
                    adjt_t = adjp.tile([P, N], BF16, tag="adj")
                    nc.sync.dma_start(adjt_t, adjt_d[jt * P:(jt + 1) * P, :])
                    # L'[j,i] = (f1'[i] + f2'[j]) * adjT[j,i]
                    lt = lp.tile([P, N], BF16, tag="lt")
                    nc.vector.scalar_tensor_tensor(
                        lt, f1b, f2cb[:, jt:jt + 1], adjt_t,
                        op0=ALU.add, op1=ALU.mult,
                    )
                    # s' = sigmoid(L' + C), accum -> per-j partial colsum
                    sp = etp.tile([P, N], F32R, tag="sp")
                    cs = smalls.tile([P, 1], F32, tag="cs")
                    nc.scalar.activation(sp, lt, AF.Sigmoid, bias=cbias, accum_out=cs)
                    if pend is not None:
                        post_sigma(*pend)
                    pend = (jt, cs, sp)
                post_sigma(*pend)

                out_sb = singles.tile([COUT, N], F32)
                for c in range(NIC):
                    csl = slice(c * 512, (c + 1) * 512)
                    eng = nc.vector if c % 2 == 0 else nc.scalar
                    if c % 2 == 0:
                        nc.vector.tensor_copy(out_sb[:, csl], ps_out[:, csl])
                    else:
                        nc.scalar.copy(out_sb[:, csl], ps_out[:, csl])
                    nc.sync.dma_start(outp_d[:, csl], out_sb[:, csl])
                nc.sync.dma_start(gout_d[:, :], g_all)

    nc.finalize()
    return nc


def _prep_in_maps(node_rep, adj_matrix, node_type, proj_W, proj_b, k_W, k_b, v_W, v_b):
    """Host-side shard prep (layout/cast/gather only, no model math)."""
    f32 = np.float32
    bf = ml_dtypes.bfloat16
    node_rep = np.asarray(node_rep, dtype=f32)
    adj = np.asarray(adj_matrix, dtype=f32)
    nt = np.asarray(node_type).astype(np.int64) % 5
    proj_W = np.asarray(proj_W, dtype=f32)
    proj_b = np.asarray(proj_b, dtype=f32)
    k_W = np.asarray(k_W, dtype=f32) * f32(FIT_B)
    k_b = np.asarray(k_b, dtype=f32) * f32(FIT_B)
    v_W = np.asarray(v_W, dtype=f32) * f32(FIT_B)
    v_b = np.asarray(v_b, dtype=f32) * f32(FIT_B)

    adjT = np.ascontiguousarray(adj.T.astype(bf))            # [j, i] bf16
    wpt = np.ascontiguousarray(proj_W.T.astype(bf))          # [CIN, COUT]
    bpcol = np.ascontiguousarray(proj_b[:, None])            # [COUT, 1]
    bpb = np.ascontiguousarray(np.broadcast_to(proj_b[None, :], (P, COUT)))
    kwt = np.ascontiguousarray(k_W[nt].T.astype(bf))         # [COUT, N]
    kbrow = np.ascontiguousarray(k_b[nt][None, :].astype(bf))  # [1, N]
    VW = v_W[nt]                                             # [N, COUT]
    vb = v_b[nt]                                             # [N]

    in_maps = []
    for core in range(8):
        b, half = divmod(core, 2)
        jsl = slice(half * NJ, (half + 1) * NJ)
        xT = np.ascontiguousarray(node_rep[b].T.astype(bf))  # [CIN, N]
        vw_h = VW[jsl]                                       # [NJ, COUT]
        vwn = np.ascontiguousarray(
            vw_h.reshape(NJT, P, COUT).transpose(1, 0, 2).reshape(P, NJT * COUT)
            .astype(bf)
        )
        vbcol = np.ascontiguousarray(vb[jsl].reshape(NJT, P).T)  # [P, NJT]
        in_maps.append({
            "adjt": np.ascontiguousarray(adjT[jsl, :]),
            "xt": xT,
            "xth": np.ascontiguousarray(xT[:, jsl]),
            "wpt": wpt,
            "bpcol": bpcol,
            "bpb": bpb,
            "kwt": kwt,
            "kbrow": kbrow,
            "vwn": vwn,
            "vbcol": vbcol,
            "ones65": np.ones((COUT + 1, P), dtype=bf),
        })
    return in_maps


def kernel(node_rep, adj_matrix, node_type, proj_W, proj_b, k_W, k_b, v_W, v_b):
    global LAST_EXEC_NS, LAST_RESULTS
    in_maps = _prep_in_maps(
        node_rep, adj_matrix, node_type, proj_W, proj_b, k_W, k_b, v_W, v_b
    )
    nc = build_nc()
    trace = os.environ.get("KERNEL_TRACE", "0") == "1"
    res = run_bass_kernel_spmd(nc, in_maps, core_ids=list(range(8)), trace=trace)
    LAST_EXEC_NS = res.exec_time_ns
    LAST_RESULTS = res

    out = np.empty((B, N, COUT), dtype=np.float32)
    for b in range(B):
        m = None
        sg = None
        for half in range(2):
            r = res.results[2 * b + half]
            mp = np.asarray(r["outp"], dtype=np.float32)          # [COUT, N]
            gp = np.asarray(r["gout"], dtype=np.float32)          # [P, NJT*COUT]
            sp = gp.reshape(P, NJT, COUT).sum(axis=(0, 1))        # [COUT]
            m = mp if m is None else m + mp
            sg = sp if sg is None else sg + sp
        out[b] = FIT_A * m.T + FIT_D * sg[None, :]
    return out
